# revision 22
# baseline (speedup 1.0000x reference)
"""CrossModalMamba Trainium2 kernel.

Sharding: 8 cores = (batch 4) x (direction 2). Direction 0 = t2i (text out),
direction 1 = i2t (image out). Each core computes one full (batch, direction)
stream: projections, cross-attention, selective scan, output projection + LN.

Layout: activations transposed [d, t] so matmuls contract over partitions.
Attention scores are produced directly transposed ([i, t] tiles via
lhsT=kT, rhs=qT), so no PE transposes are needed; the softmax denominator is
folded into the tiny [2N, t] B/C projection via a ones-matmul partition
reduction. The selective scan (DVE TensorTensorScan over a free axis ordered
(n, t)) for chunk c runs concurrently with attention for chunk c+1; the
output projection for chunk c overlaps the scan of chunk c+1.
"""
import numpy as np

import concourse.bass as bass
import concourse.bacc as bacc
import concourse.mybir as mybir
import concourse.tile as tile

F16 = mybir.dt.float16
F32 = mybir.dt.float32
AF = mybir.ActivationFunctionType
OP = mybir.AluOpType

B, L, D, N = 4, 1024, 512, 16
DT = D // 128          # 4 d-tiles
TTN = L // 128         # 8 i-tiles
TC = 512               # scan time-chunk
NCH = L // TC          # 2 chunks
EPS = 1e-5
SCALE = 1.0 / np.sqrt(np.float32(D))
NS = 9                 # dBx rows on DVE (rest on GpSimd)
YS = 10                # hC rows on DVE (rest on GpSimd)

_CACHE = {}


def _pin_act_tables():
    import functools
    import concourse.hw_specs as hw_specs
    import concourse.bacc as _bacc
    orig = hw_specs.get_activation_tables.__wrapped__

    @functools.cache
    def pinned(module_arch):
        # Keep every entry (set ids must stay aligned with walrus's
        # act_info.json order) but empty all sets except the one table that
        # covers Exp/Ln/Identity/Copy, so the chooser never switches tables.
        tabs = orig(module_arch)
        keep = "natural_log_exp_and_others"
        return {k: (v if k == keep else set()) for k, v in tabs.items()}

    hw_specs.get_activation_tables = pinned
    _bacc.get_activation_tables = pinned


def _build_program(zero_bias=False, unit_ln=False):
    if not _CACHE.get("_pinned"):
        _pin_act_tables()
        _CACHE["_pinned"] = True
    nc = bacc.Bacc(None, target_bir_lowering=False)

    xs_d = nc.dram_tensor("xT_self", [D, L], F16, kind="ExternalInput")
    xo_d = nc.dram_tensor("xT_other", [D, L], F16, kind="ExternalInput")
    wq_d = nc.dram_tensor("w_q", [D, D], F16, kind="ExternalInput")
    wk_d = nc.dram_tensor("w_k", [D, D], F16, kind="ExternalInput")
    wdt_d = nc.dram_tensor("w_dt", [D, D], F16, kind="ExternalInput")
    wbc_d = nc.dram_tensor("w_bc", [D, 2 * N], F16, kind="ExternalInput")
    wout_d = nc.dram_tensor("w_out", [2 * D, D], F16, kind="ExternalInput")
    acols_d = nc.dram_tensor("a_cols", [128, DT * N], F32, kind="ExternalInput")
    bq_d = nc.dram_tensor("b_q", [128, DT], F32, kind="ExternalInput")
    bdt_d = nc.dram_tensor("b_dt", [128, DT], F32, kind="ExternalInput")
    bk_d = nc.dram_tensor("b_k", [128, DT], F32, kind="ExternalInput")
    bkrow_d = nc.dram_tensor("b_k_row", [1, D], F16, kind="ExternalInput")
    bbc_d = nc.dram_tensor("b_bc", [2 * N, 1], F32, kind="ExternalInput")
    bout_d = nc.dram_tensor("b_out_row", [1, D], F16, kind="ExternalInput")
    gam_d = nc.dram_tensor("gamma_row", [1, D], F32, kind="ExternalInput")
    bet_d = nc.dram_tensor("beta_row", [1, D], F32, kind="ExternalInput")
    ones_d = nc.dram_tensor("ones_row", [1, 128], F16, kind="ExternalInput")
    id_d = nc.dram_tensor("ident", [128, 128], F16, kind="ExternalInput")
    out_d = nc.dram_tensor("out", [L, D], F16, kind="ExternalOutput")

    with tile.TileContext(nc) as tc:
        with (
            tc.tile_pool(name="wp", bufs=1) as wp,
            tc.tile_pool(name="pp", bufs=1) as pp,
            tc.tile_pool(name="sm", bufs=2) as sm,
            tc.tile_pool(name="psA", bufs=3, space="PSUM") as psA,
            tc.tile_pool(name="psR", bufs=1, space="PSUM") as psR,
            tc.tile_pool(name="psY", bufs=2, space="PSUM") as psY,
            tc.tile_pool(name="dramp", bufs=1, space="DRAM") as dramp,
        ):
            # ---- constants ----
            acols = wp.tile([128, DT * N], F32, tag="acols", name="acols")
            bq = wp.tile([128, DT], F32, tag="bq", name="bq")
            bdt = wp.tile([128, DT], F32, tag="bdt", name="bdt")
            bk = wp.tile([128, DT], F32, tag="bk", name="bk")
            bbc = wp.tile([2 * N, 1], F32, tag="bbc", name="bbc")
            if not zero_bias:
                bkrow = wp.tile([1, D], F16, tag="bkrow", name="bkrow")
                bout = wp.tile([1, D], F16, tag="bout", name="bout")
                ones_r = wp.tile([1, 128], F16, tag="ones", name="ones")
            else:
                bkrow = bout = ones_r = None
            onef = wp.tile([128, 1], F32, tag="onef", name="onef")
            nc.vector.memset(onef, 1.0)
            ones32 = wp.tile([128, 2 * N], F16, tag="ones32", name="ones32")
            nc.vector.memset(ones32, 1.0)
            idt = wp.tile([128, 128], F16, tag="idt", name="idt")

            def _load_constants():
                nc.sync.dma_start(out=acols, in_=acols_d[:, :])
                nc.sync.dma_start(out=bq, in_=bq_d[:, :])
                nc.sync.dma_start(out=bdt, in_=bdt_d[:, :])
                nc.sync.dma_start(out=bk, in_=bk_d[:, :])
                nc.sync.dma_start(out=bbc, in_=bbc_d[:, :])
                nc.sync.dma_start(out=idt, in_=id_d[:, :])
                if not zero_bias:
                    nc.sync.dma_start(out=bkrow, in_=bkrow_d[:, :])
                    nc.sync.dma_start(out=bout, in_=bout_d[:, :])
                    nc.sync.dma_start(out=ones_r, in_=ones_d[:, :])

            # ---- persistent activations ----
            x_s = [pp.tile([128, L], F16, tag=f"xs{m}", name=f"xs{m}") for m in range(DT)]
            spdt = [pp.tile([128, L], F16, tag=f"sp{m}", name=f"sp{m}") for m in range(DT)]
            u = [pp.tile([128, L], F16, tag=f"u{m}", name=f"u{m}") for m in range(DT)]
            yT = [pp.tile([128, L], F16, tag=f"yT{m}", name=f"yT{m}") for m in range(DT)]
            wout = [pp.tile([128, D], F16, tag=f"wout{k}", name=f"wout{k}")
                    for k in range(2 * DT)]

            for m in range(DT):
                nc.sync.dma_start(out=x_s[m], in_=xs_d[m * 128:(m + 1) * 128, :])

            pre_dA = {}

            def proj_half(dst, wsrc, xsrc, bcol, half):
                """dst[:, half] = W^T x for one 512-col half, all m-tiles."""
                sl = slice(half * 512, (half + 1) * 512)
                for m in range(DT):
                    msl = slice(m * 128, (m + 1) * 128)
                    ps = psA.tile([128, 512], F32, tag="ps", name="proj")
                    for k in range(DT):
                        nc.tensor.matmul(ps, lhsT=wsrc[k][:, msl],
                                         rhs=xsrc[k][:, sl],
                                         start=(k == 0), stop=(k == DT - 1))
                    if zero_bias:
                        nc.vector.tensor_copy(out=dst[m][:, sl], in_=ps)
                    else:
                        nc.vector.tensor_scalar(out=dst[m][:, sl], in0=ps,
                                                scalar1=bcol[:, m:m + 1],
                                                scalar2=None, op0=OP.add)

            def dt_half(ph1, wdt, half):
                """softplus(dt) and u for one 512-col half, all m-tiles."""
                sl = slice(half * 512, (half + 1) * 512)
                for m in range(DT):
                    msl = slice(m * 128, (m + 1) * 128)
                    # softplus(x) = ln(exp(x) + 1); |x| <= ~3 so no overflow.
                    edt = ph1.tile([128, 512], F32, tag="edt", name=f"edt{m}",
                                   bufs=1)
                    psd = psA.tile([128, 512], F32, tag="ps", name="proj")
                    for k in range(DT):
                        nc.tensor.matmul(psd, lhsT=wdt[k][:, msl],
                                         rhs=qT[k][:, sl],
                                         start=(k == 0), stop=(k == DT - 1))
                    nc.scalar.activation(out=edt, in_=psd, func=AF.Exp,
                                         bias=bdt[:, m:m + 1])
                    nc.scalar.activation(out=spdt[m][:, sl], in_=edt, func=AF.Ln,
                                         bias=onef)
                    nc.vector.tensor_mul(u[m][:, sl], spdt[m][:, sl],
                                         qT[m][:, sl])

            def knat_tr():
                # k natural layout [i, dk] via PE transposes of kT (the k
                # projection already includes its bias)
                for i in range(TTN):
                    isl = slice(i * 128, (i + 1) * 128)
                    pst = psY.tile([128, 512], F16, tag="tr", name="tr")
                    for k in range(DT):
                        nc.tensor.transpose(pst[:, k * 128:(k + 1) * 128],
                                            kT[k][:, isl], idt)
                    nc.scalar.copy(out=k_nat[i], in_=pst)

            def attn_chunk(c, wbc, mid_hook=None):
                sl = slice(c * 512, (c + 1) * 512)
                # scores directly transposed: [i-part, t-free]
                for i in range(TTN):
                    isl = slice(i * 128, (i + 1) * 128)
                    psx = psA.tile([128, 512], F32, tag="ps", name="qk")
                    for k in range(DT):
                        nc.tensor.matmul(psx, lhsT=kT[k][:, isl],
                                         rhs=qT[k][:, sl],
                                         start=(k == 0), stop=(k == DT - 1))
                    nc.scalar.activation(out=attnT[i], in_=psx, func=AF.Exp,
                                         scale=float(SCALE))
                if mid_hook is not None:
                    mid_hook()
                # softmax denominator, replicated onto 2N partitions
                psr = psR.tile([2 * N, 512], F32, tag="rs", name="rs")
                for i in range(TTN):
                    nc.tensor.matmul(psr, lhsT=ones32, rhs=attnT[i],
                                     start=(i == 0), stop=(i == TTN - 1))
                rsr = sm.tile([2 * N, 512], F16, tag="rsr", name="rsr", bufs=1)
                with nc.allow_low_precision(reason="1/softmax-denominator in f16 is ~5e-4 rel"):
                    nc.vector.reciprocal(rsr, psr)
                # B_feat (unnormalized)
                for m in range(DT):
                    msl = slice(m * 128, (m + 1) * 128)
                    ps = psA.tile([128, 512], F32, tag="ps", name="bf")
                    for i in range(TTN):
                        nc.tensor.matmul(ps, lhsT=k_nat[i][:, msl],
                                         rhs=attnT[i],
                                         start=(i == 0), stop=(i == TTN - 1))
                    nc.scalar.copy(out=bfT[m], in_=ps)
                # B/C projection, scaled by the softmax denominator
                ps = psA.tile([2 * N, 512], F32, tag="ps", name="bc")
                for k in range(DT):
                    nc.tensor.matmul(ps, lhsT=wbc[k], rhs=bfT[k],
                                     start=(k == 0), stop=(k == DT - 1))
                nc.vector.tensor_mul(bc_nT[:, sl], ps, rsr)
                if not zero_bias:
                    nc.vector.tensor_scalar(out=bc_nT[:, sl], in0=bc_nT[:, sl],
                                            scalar1=bbc[:, 0:1], scalar2=None,
                                            op0=OP.add)
                nc.sync.dma_start(out=bc_dram[:, sl], in_=bc_nT[:, sl])

            def bc_broadcast(c):
                sl = slice(c * 512, (c + 1) * 512)
                nc.sync.dma_start(
                    out=Bb[c],
                    in_=bc_dram[0:N, sl].unsqueeze(0).broadcast_to([128, N, TC]))
                nc.sync.dma_start(
                    out=Cb[c],
                    in_=bc_dram[N:2 * N, sl].unsqueeze(0).broadcast_to([128, N, TC]))

            def dA_exp(c, ms, scp):
                sl = slice(c * TC, (c + 1) * TC)
                for m in ms:
                    dA = scp.tile([128, N, TC], F16, tag="dA", name="dA", bufs=3)
                    pre_dA[(c, m)] = dA
                    for n in range(N):
                        nc.scalar.activation(
                            out=dA[:, n, :], in_=spdt[m][:, sl], func=AF.Exp,
                            scale=acols[:, m * N + n:m * N + n + 1])

            def scan_chunk(c, scp, hook=None):
                sl = slice(c * TC, (c + 1) * TC)
                for m in range(DT):
                    if hook is not None:
                        hook(m)
                    dA = pre_dA.pop((c, m))
                    dBx = scp.tile([128, N, TC], F16, tag="dBx", name="dBx",
                                   bufs=3)
                    uv = u[m][:, sl].unsqueeze(1).broadcast_to([128, N, TC])
                    # split row-wise between DVE (2x mode) and GpSimd so both
                    # finish together
                    nc.vector.tensor_mul(dBx[:, :NS], uv[:, :NS], Bb[c][:, :NS])
                    nc.gpsimd.tensor_mul(dBx[:, NS:], uv[:, NS:], Bb[c][:, NS:])
                    if c == 0:
                        nc.vector.memset(dA[:, :, 0:1], 0.0)
                    else:
                        tmp = sm.tile([128, N, 1], F16, tag="carry", name="carry")
                        nc.vector.tensor_mul(tmp, dA[:, :, 0:1], hcar[m])
                        nc.vector.tensor_add(dBx[:, :, 0:1], dBx[:, :, 0:1], tmp)
                        nc.vector.memset(dA[:, :, 0:1], 0.0)
                    # scan in place: h overwrites dA (dA's last access -> its
                    # buffer frees at scan end, keeping the ACT exp pipeline
                    # ahead of DVE)
                    nc.vector.tensor_tensor_scan(
                        out=dA.rearrange("p n t -> p (n t)"),
                        data0=dA.rearrange("p n t -> p (n t)"),
                        data1=dBx.rearrange("p n t -> p (n t)"),
                        initial=0.0, op0=OP.mult, op1=OP.add)
                    if c < NCH - 1:
                        nc.vector.tensor_copy(out=hcar[m], in_=dA[:, :, TC - 1:TC])
                    # hC overwrites the dead dBx tile, not h: frees dA early
                    nc.vector.tensor_mul(dBx[:, :YS], dA[:, :YS], Cb[c][:, :YS])
                    nc.gpsimd.tensor_mul(dBx[:, YS:], dA[:, YS:], Cb[c][:, YS:])
                    psy = psY.tile([128, TC], F32, tag="ys", name="ys")
                    for n in range(N):
                        nc.tensor.matmul(psy, lhsT=idt, rhs=dBx[:, n, :],
                                         start=(n == 0), stop=(n == N - 1))
                    nc.scalar.copy(out=yT[m][:, sl], in_=psy)

            ln_pend = []

            def outproj_chunk(c, op, gam, bet, epst):
                for t in range(4 * c, 4 * c + 4):
                    tsl = slice(t * 128, (t + 1) * 128)
                    ps = psA.tile([128, D], F32, tag="ps", name=f"out{t}")
                    for k in range(DT):
                        nc.tensor.matmul(ps, lhsT=x_s[k][:, tsl], rhs=wout[k],
                                         start=(k == 0), stop=False)
                    for k in range(DT):
                        nc.tensor.matmul(ps, lhsT=yT[k][:, tsl], rhs=wout[DT + k],
                                         start=False,
                                         stop=(zero_bias and k == DT - 1))
                    if not zero_bias:
                        nc.tensor.matmul(ps, lhsT=ones_r, rhs=bout, start=False,
                                         stop=True)
                    outs = op.tile([128, D], F16, tag="outs", name=f"outs{t}",
                                   bufs=4)
                    sx = op.tile([128, 1], F32, tag=f"sx{t}", name=f"sx{t}", bufs=1)
                    nc.scalar.activation(out=outs, in_=ps, func=AF.Copy,
                                         accum_out=sx)
                    sq = op.tile([128, D], F16, tag="sq", name="sq", bufs=1)
                    sx2 = op.tile([128, 1], F32, tag=f"sx2{t}", name=f"sx2{t}",
                                  bufs=1)
                    nc.scalar.activation(out=sq, in_=outs, func=AF.Square,
                                         accum_out=sx2)
                    ln_pend.append((t, outs, sx, sx2))

            def ln_finish(op, gam, bet, epst):
                # mean = sx/D; var = sx2/D - mean^2 (benign cancellation);
                # rstd = exp(-0.5*ln(var+eps))
                pend, ln_pend[:] = list(ln_pend), []
                for t, outs, sx, sx2 in pend:
                    mean = op.tile([128, 1], F32, tag="mean", name="mean")
                    nc.vector.tensor_scalar(out=mean, in0=sx,
                                            scalar1=1.0 / D, scalar2=None,
                                            op0=OP.mult)
                    m2 = op.tile([128, 1], F32, tag="m2", name="m2")
                    nc.vector.tensor_mul(m2, mean, mean)
                    var = op.tile([128, 1], F32, tag="var", name="var")
                    nc.vector.tensor_scalar(out=var, in0=sx2,
                                            scalar1=1.0 / D, scalar2=m2,
                                            op0=OP.mult, op1=OP.subtract)
                    rstd = op.tile([128, 1], F32, tag="rstd", name="rstd")
                    nc.scalar.activation(out=rstd, in_=var, func=AF.Ln,
                                         bias=epst)
                    nc.scalar.activation(out=rstd, in_=rstd, func=AF.Exp,
                                         scale=-0.5)
                    norm = op.tile([128, D], F16, tag="norm", name="norm",
                                   bufs=2)
                    nc.vector.tensor_scalar(out=norm, in0=outs,
                                            scalar1=mean, scalar2=rstd,
                                            op0=OP.subtract, op1=OP.mult)
                    if not unit_ln:
                        nc.gpsimd.tensor_mul(norm, norm, gam)
                        nc.gpsimd.tensor_add(norm, norm, bet)
                    tsl = slice(t * 128, (t + 1) * 128)
                    nc.sync.dma_start(out=out_d[tsl, :], in_=norm)

            # =================== program ===================
            bc_dram = dramp.tile([2 * N, L], F16)

            with (
                tc.tile_pool(name="scp", bufs=1) as scp,
                tc.tile_pool(name="op", bufs=4) as op,
            ):
                hcar = [scp.tile([128, N, 1], F16, tag=f"hcar{m}", name=f"hcar{m}")
                        for m in range(DT)]
                Bb = [scp.tile([128, N, TC], F16, tag=f"Bb{c}", name=f"Bb{c}")
                      for c in range(NCH)]
                Cb = [scp.tile([128, N, TC], F16, tag="Cb", name=f"Cb{c}")
                      for c in range(NCH)]
                if not unit_ln:
                    gam = op.tile([128, D], F32, tag="gam", name="gam", bufs=1)
                    bet = op.tile([128, D], F32, tag="bet", name="bet", bufs=1)
                else:
                    gam = bet = None
                epst = op.tile([128, 1], F32, tag="eps", name="eps", bufs=1)
                nc.vector.memset(epst, EPS)

                with tc.tile_pool(name="phA", bufs=1) as phA, \
                        tc.tile_pool(name="ph1", bufs=1) as ph1:
                    qT = [phA.tile([128, L], F16, tag=f"qT{m}", name=f"qT{m}") for m in range(DT)]
                    kT = [phA.tile([128, L], F16, tag=f"kT{m}", name=f"kT{m}") for m in range(DT)]
                    k_nat = [phA.tile([128, D], F16, tag=f"kn{i}", name=f"kn{i}") for i in range(TTN)]
                    # attnT[i]: unnormalized exp(scores) laid out [i-part,
                    # t-free], one 512-wide chunk slice at a time
                    attnT = [phA.tile([128, TC], F16, tag=f"aT{i}", name=f"aT{i}")
                             for i in range(TTN)]
                    bfT = [phA.tile([128, TC], F16, tag=f"bf{m}", name=f"bf{m}")
                           for m in range(DT)]
                    bc_nT = phA.tile([2 * N, L], F16, tag="bcnT", name="bcnT")
                    x_o = [ph1.tile([128, L], F16, tag=f"xo{m}", name=f"xo{m}")
                           for m in range(DT)]
                    for m in range(DT):
                        nc.sync.dma_start(out=x_o[m], in_=xo_d[m * 128:(m + 1) * 128, :])
                    wq = [ph1.tile([128, D], F16, tag=f"wq{k}", name=f"wq{k}") for k in range(DT)]
                    wk = [ph1.tile([128, D], F16, tag=f"wk{k}", name=f"wk{k}") for k in range(DT)]
                    wdt = [ph1.tile([128, D], F16, tag=f"wdt{k}", name=f"wdt{k}") for k in range(DT)]
                    wbc = [wp.tile([128, 2 * N], F16, tag=f"wbc{k}", name=f"wbc{k}") for k in range(DT)]
                    for k in range(DT):
                        nc.sync.dma_start(out=wk[k], in_=wk_d[k * 128:(k + 1) * 128, :])
                        nc.sync.dma_start(out=wq[k], in_=wq_d[k * 128:(k + 1) * 128, :])
                        nc.sync.dma_start(out=wdt[k], in_=wdt_d[k * 128:(k + 1) * 128, :])
                        nc.sync.dma_start(out=wbc[k], in_=wbc_d[k * 128:(k + 1) * 128, :])
                    _load_constants()
                    if not unit_ln:
                        nc.sync.dma_start(out=gam, in_=gam_d[0:1, :].broadcast_to([128, D]))
                        nc.sync.dma_start(out=bet, in_=bet_d[0:1, :].broadcast_to([128, D]))
                    for k in range(2 * DT):
                        nc.sync.dma_start(out=wout[k],
                                          in_=wout_d[k * 128:(k + 1) * 128, :])

                    proj_half(kT, wk, x_o, bk, 0)
                    proj_half(kT, wk, x_o, bk, 1)
                    proj_half(qT, wq, x_s, bq, 0)
                    attn_chunk(0, wbc,
                               mid_hook=lambda: (dt_half(ph1, wdt, 0),
                                                 knat_tr()))
                    bc_broadcast(0)
                    dA_exp(0, [0], scp)
                    proj_half(qT, wq, x_s, bq, 1)
                    dA_exp(0, [1], scp)
                    dt_half(ph1, wdt, 1)
                    attn_chunk(1, wbc)
                    bc_broadcast(1)
                    dA_exp(0, [2, 3], scp)

                with tc.tile_pool(name="scp2", bufs=1) as scp2:
                    scan_chunk(0, scp2, hook=lambda m: dA_exp(1, [m], scp))
                    outproj_chunk(0, op, gam, bet, epst)
                    scan_chunk(1, scp2)
                    ln_finish(op, gam, bet, epst)
                    outproj_chunk(1, op, gam, bet, epst)
                    ln_finish(op, gam, bet, epst)
    nc.finalize()
    return nc


def _make_runner(nc, n_cores):
    import jax
    from concourse import bass2jax

    bass2jax.install_neuronx_cc_hook()
    partition_name = nc.partition_id_tensor.name if nc.partition_id_tensor else None
    in_names, out_names, out_avals, zero_outs = [], [], [], []
    for alloc in nc.m.functions[0].allocations:
        if not isinstance(alloc, mybir.MemoryLocationSet):
            continue
        name = alloc.memorylocations[0].name
        if alloc.kind == "ExternalInput":
            if name != partition_name:
                in_names.append(name)
        elif alloc.kind == "ExternalOutput":
            shape = tuple(alloc.tensor_shape)
            dtype = mybir.dt.np(alloc.dtype)
            out_names.append(name)
            out_avals.append(jax.core.ShapedArray(shape, dtype))
            zero_outs.append(np.zeros(shape, dtype))
    all_in_names = list(in_names) + list(out_names)
    if partition_name is not None:
        all_in_names.append(partition_name)

    def _body(*args):
        operands = list(args)
        if partition_name is not None:
            operands.append(bass2jax.partition_id_tensor())
        outs = bass2jax._bass_exec_p.bind(
            *operands,
            out_avals=tuple(out_avals),
            in_names=tuple(all_in_names),
            out_names=tuple(out_names),
            lowering_input_output_aliases=(),
            sim_require_finite=False,
            sim_require_nnan=False,
            nc=nc,
        )
        return tuple(outs)

    if n_cores == 1:
        jit_body = jax.jit(_body, keep_unused=True)

        def run(in_maps):
            args = [np.asarray(in_maps[0][n]) for n in in_names] + zero_outs
            outs = jit_body(*args)
            return [dict(zip(out_names, [np.asarray(o) for o in outs]))]
        return run

    from jax.sharding import PartitionSpec as P
    from jax.experimental.shard_map import shard_map
    mesh = jax.make_mesh((n_cores,), ("core",), devices=jax.devices()[:n_cores])
    smapped = jax.jit(
        shard_map(_body, mesh=mesh, in_specs=P("core"), out_specs=P("core"),
                  check_rep=False),
        keep_unused=True)

    def run(in_maps):
        assert len(in_maps) == n_cores
        args = [np.concatenate([np.asarray(m[n]) for m in in_maps], axis=0)
                for n in in_names]
        args += [np.concatenate([z] * n_cores, axis=0) for z in zero_outs]
        outs = [np.asarray(o) for o in smapped(*args)]
        percore = []
        for cidx in range(n_cores):
            d = {}
            for name, o in zip(out_names, outs):
                rows = o.shape[0] // n_cores
                d[name] = o[cidx * rows:(cidx + 1) * rows]
            percore.append(d)
        return percore
    return run


def _core_inputs(x_self, x_other, W_self, b_self, W_other, b_other,
                 W_dt, b_dt, W_B, b_B, W_C, b_C, A, W_out, b_out, gamma, beta):
    f16 = np.float16
    f32 = np.float32
    return {
        "xT_self": np.ascontiguousarray(x_self.T).astype(f16),
        "xT_other": np.ascontiguousarray(x_other.T).astype(f16),
        "w_q": np.ascontiguousarray(W_self[:, :D]).astype(f16),
        "w_k": np.ascontiguousarray(W_other[:, D:]).astype(f16),
        "w_dt": np.ascontiguousarray(W_dt).astype(f16),
        "w_bc": np.concatenate([W_B, W_C], axis=1).astype(f16),
        "w_out": np.ascontiguousarray(W_out).astype(f16),
        "a_cols": np.ascontiguousarray(
            A.reshape(DT, 128, N).transpose(1, 0, 2).reshape(128, DT * N)).astype(f32),
        "b_q": np.ascontiguousarray(b_self[:D].reshape(DT, 128).T).astype(f32),
        "b_dt": np.ascontiguousarray(b_dt.reshape(DT, 128).T).astype(f32),
        "b_k": np.ascontiguousarray(b_other[D:].reshape(DT, 128).T).astype(f32),
        "b_k_row": b_other[D:].reshape(1, D).astype(f16),
        "b_bc": np.concatenate([b_B, b_C]).reshape(2 * N, 1).astype(f32),
        "b_out_row": b_out.reshape(1, D).astype(f16),
        "gamma_row": gamma.reshape(1, D).astype(f32),
        "beta_row": beta.reshape(1, D).astype(f32),
        "ones_row": np.ones((1, 128), f16),
        "ident": np.eye(128, dtype=f16),
    }


def kernel(text_feats, image_feats,
           W_t2i, b_t2i, W_i2t, b_i2t,
           W_dt_t2i, b_dt_t2i, W_dt_i2t, b_dt_i2t,
           W_B_t2i, b_B_t2i, W_B_i2t, b_B_i2t,
           W_C_t2i, b_C_t2i, W_C_i2t, b_C_i2t,
           A_log, W_out, b_out, gamma, beta):
    text_feats = np.asarray(text_feats, np.float32)
    image_feats = np.asarray(image_feats, np.float32)
    A = -np.exp(np.asarray(A_log, np.float32))

    zero_bias = all(not np.any(np.asarray(x)) for x in
                    (b_t2i, b_i2t, b_dt_t2i, b_dt_i2t, b_B_t2i, b_B_i2t,
                     b_C_t2i, b_C_i2t, b_out))
    unit_ln = (not np.any(np.asarray(beta))) and np.all(np.asarray(gamma) == 1.0)
    key = ("nc", zero_bias, unit_ln)
    if key not in _CACHE:
        _CACHE[key] = _build_program(zero_bias=zero_bias, unit_ln=unit_ln)
        _CACHE["nc"] = _CACHE[key]
    nc = _CACHE[key]
    n_cores = 8
    if ("runner", key) not in _CACHE:
        _CACHE[("runner", key)] = _make_runner(nc, n_cores)
    run = _CACHE[("runner", key)]

    in_maps = []
    for b in range(B):
        # direction 0: t2i (text output)
        in_maps.append(_core_inputs(
            text_feats[b], image_feats[b], W_t2i, b_t2i, W_i2t, b_i2t,
            W_dt_t2i, b_dt_t2i, W_B_t2i, b_B_t2i, W_C_t2i, b_C_t2i,
            A, W_out, b_out, gamma, beta))
        # direction 1: i2t (image output)
        in_maps.append(_core_inputs(
            image_feats[b], text_feats[b], W_i2t, b_i2t, W_t2i, b_t2i,
            W_dt_i2t, b_dt_i2t, W_B_i2t, b_B_i2t, W_C_i2t, b_C_i2t,
            A, W_out, b_out, gamma, beta))

    results = run(in_maps)
    text_out = np.stack([results[2 * b]["out"] for b in range(B)],
                        axis=0).astype(np.float32)
    image_out = np.stack([results[2 * b + 1]["out"] for b in range(B)],
                         axis=0).astype(np.float32)
    return text_out, image_out


# revision 23
# speedup vs baseline: 1.0796x; 1.0796x over previous
"""CrossModalMamba Trainium2 kernel.

Sharding: 8 cores = (batch 4) x (direction 2). Direction 0 = t2i (text out),
direction 1 = i2t (image out). Each core computes one full (batch, direction)
stream: projections, cross-attention, selective scan, output projection + LN.

Layout: activations transposed [d, t] so matmuls contract over partitions.
Attention scores are produced directly transposed ([i, t] tiles via
lhsT=kT, rhs=qT), so no PE transposes are needed; the softmax denominator is
folded into the tiny [2N, t] B/C projection via a ones-matmul partition
reduction. The selective scan (DVE TensorTensorScan over a free axis ordered
(n, t)) for chunk c runs concurrently with attention for chunk c+1; the
output projection for chunk c overlaps the scan of chunk c+1.
"""
import numpy as np

import concourse.bass as bass
import concourse.bacc as bacc
import concourse.mybir as mybir
import concourse.tile as tile

F16 = mybir.dt.float16
F32 = mybir.dt.float32
AF = mybir.ActivationFunctionType
OP = mybir.AluOpType

B, L, D, N = 4, 1024, 512, 16
DT = D // 128          # 4 d-tiles
TTN = L // 128         # 8 i-tiles
TC = 512               # scan time-chunk
NCH = L // TC          # 2 chunks
EPS = 1e-5
SCALE = 1.0 / np.sqrt(np.float32(D))
NS = 9                 # dBx rows on DVE (rest on GpSimd)
YS = 10                # hC rows on DVE (rest on GpSimd)

_CACHE = {}


def _pin_act_tables():
    import functools
    import concourse.hw_specs as hw_specs
    import concourse.bacc as _bacc
    orig = hw_specs.get_activation_tables.__wrapped__

    @functools.cache
    def pinned(module_arch):
        # Keep every entry (set ids must stay aligned with walrus's
        # act_info.json order) but empty all sets except the one table that
        # covers Exp/Ln/Identity/Copy, so the chooser never switches tables.
        tabs = orig(module_arch)
        keep = "natural_log_exp_and_others"
        return {k: (v if k == keep else set()) for k, v in tabs.items()}

    hw_specs.get_activation_tables = pinned
    _bacc.get_activation_tables = pinned


def _build_program(zero_bias=False, unit_ln=False):
    if not _CACHE.get("_pinned"):
        _pin_act_tables()
        _CACHE["_pinned"] = True
    nc = bacc.Bacc(None, target_bir_lowering=False)

    xs_d = nc.dram_tensor("xT_self", [D, L], F16, kind="ExternalInput")
    xo_d = nc.dram_tensor("xT_other", [D, L], F16, kind="ExternalInput")
    wq_d = nc.dram_tensor("w_q", [D, D], F16, kind="ExternalInput")
    wk_d = nc.dram_tensor("w_k", [D, D], F16, kind="ExternalInput")
    wdt_d = nc.dram_tensor("w_dt", [D, D], F16, kind="ExternalInput")
    wbc_d = nc.dram_tensor("w_bc", [D, 2 * N], F16, kind="ExternalInput")
    wout_d = nc.dram_tensor("w_out", [2 * D, D], F16, kind="ExternalInput")
    acols_d = nc.dram_tensor("a_cols", [128, DT * N], F32, kind="ExternalInput")
    bq_d = nc.dram_tensor("b_q", [128, DT], F32, kind="ExternalInput")
    bdt_d = nc.dram_tensor("b_dt", [128, DT], F32, kind="ExternalInput")
    bk_d = nc.dram_tensor("b_k", [128, DT], F32, kind="ExternalInput")
    bkrow_d = nc.dram_tensor("b_k_row", [1, D], F16, kind="ExternalInput")
    bbc_d = nc.dram_tensor("b_bc", [2 * N, 1], F32, kind="ExternalInput")
    bout_d = nc.dram_tensor("b_out_row", [1, D], F16, kind="ExternalInput")
    gam_d = nc.dram_tensor("gamma_row", [1, D], F32, kind="ExternalInput")
    bet_d = nc.dram_tensor("beta_row", [1, D], F32, kind="ExternalInput")
    ones_d = nc.dram_tensor("ones_row", [1, 128], F16, kind="ExternalInput")
    id_d = nc.dram_tensor("ident", [128, 128], F16, kind="ExternalInput")
    out_d = nc.dram_tensor("out", [L, D], F16, kind="ExternalOutput")

    with tile.TileContext(nc) as tc:
        with (
            tc.tile_pool(name="wp", bufs=1) as wp,
            tc.tile_pool(name="pp", bufs=1) as pp,
            tc.tile_pool(name="sm", bufs=2) as sm,
            tc.tile_pool(name="psA", bufs=3, space="PSUM") as psA,
            tc.tile_pool(name="psR", bufs=1, space="PSUM") as psR,
            tc.tile_pool(name="psY", bufs=2, space="PSUM") as psY,
            tc.tile_pool(name="dramp", bufs=1, space="DRAM") as dramp,
        ):
            # ---- constants ----
            acols = wp.tile([128, DT * N], F32, tag="acols", name="acols")
            bq = wp.tile([128, DT], F32, tag="bq", name="bq")
            bdt = wp.tile([128, DT], F32, tag="bdt", name="bdt")
            bk = wp.tile([128, DT], F32, tag="bk", name="bk")
            bbc = wp.tile([2 * N, 1], F32, tag="bbc", name="bbc")
            if not zero_bias:
                bkrow = wp.tile([1, D], F16, tag="bkrow", name="bkrow")
                bout = wp.tile([1, D], F16, tag="bout", name="bout")
                ones_r = wp.tile([1, 128], F16, tag="ones", name="ones")
            else:
                bkrow = bout = ones_r = None
            onef = wp.tile([128, 1], F32, tag="onef", name="onef")
            nc.vector.memset(onef, 1.0)
            ones32 = wp.tile([128, 2 * N], F16, tag="ones32", name="ones32")
            nc.vector.memset(ones32, 1.0)
            idt = wp.tile([128, 128], F16, tag="idt", name="idt")

            def _load_constants():
                nc.sync.dma_start(out=acols, in_=acols_d[:, :])
                nc.sync.dma_start(out=bq, in_=bq_d[:, :])
                nc.sync.dma_start(out=bdt, in_=bdt_d[:, :])
                nc.sync.dma_start(out=bk, in_=bk_d[:, :])
                nc.sync.dma_start(out=bbc, in_=bbc_d[:, :])
                nc.sync.dma_start(out=idt, in_=id_d[:, :])
                if not zero_bias:
                    nc.sync.dma_start(out=bkrow, in_=bkrow_d[:, :])
                    nc.sync.dma_start(out=bout, in_=bout_d[:, :])
                    nc.sync.dma_start(out=ones_r, in_=ones_d[:, :])

            # ---- persistent activations ----
            x_s = [pp.tile([128, L], F16, tag=f"xs{m}", name=f"xs{m}") for m in range(DT)]
            spdt = [pp.tile([128, L], F16, tag=f"sp{m}", name=f"sp{m}") for m in range(DT)]
            u = [pp.tile([128, L], F16, tag=f"u{m}", name=f"u{m}") for m in range(DT)]
            yT = [pp.tile([128, L], F16, tag=f"yT{m}", name=f"yT{m}") for m in range(DT)]
            wout = [pp.tile([128, D], F16, tag=f"wout{k}", name=f"wout{k}")
                    for k in range(2 * DT)]

            for m in range(DT):
                nc.sync.dma_start(out=x_s[m], in_=xs_d[m * 128:(m + 1) * 128, :])

            pre_dA = {}

            def proj_half(dst, wsrc, xsrc, bcol, half):
                """dst[:, half] = W^T x for one 512-col half, all m-tiles."""
                sl = slice(half * 512, (half + 1) * 512)
                for m in range(DT):
                    msl = slice(m * 128, (m + 1) * 128)
                    ps = psA.tile([128, 512], F32, tag="ps", name="proj")
                    for k in range(DT):
                        nc.tensor.matmul(ps, lhsT=wsrc[k][:, msl],
                                         rhs=xsrc[k][:, sl],
                                         start=(k == 0), stop=(k == DT - 1))
                    if zero_bias:
                        nc.vector.tensor_copy(out=dst[m][:, sl], in_=ps)
                    else:
                        nc.vector.tensor_scalar(out=dst[m][:, sl], in0=ps,
                                                scalar1=bcol[:, m:m + 1],
                                                scalar2=None, op0=OP.add)

            def dt_half(ph1, wdt, half):
                """softplus(dt) and u for one 512-col half, all m-tiles."""
                sl = slice(half * 512, (half + 1) * 512)
                for m in range(DT):
                    msl = slice(m * 128, (m + 1) * 128)
                    # softplus(x) = ln(exp(x) + 1); |x| <= ~3 so no overflow.
                    edt = ph1.tile([128, 512], F32, tag="edt", name=f"edt{m}",
                                   bufs=1)
                    psd = psA.tile([128, 512], F32, tag="ps", name="proj")
                    for k in range(DT):
                        nc.tensor.matmul(psd, lhsT=wdt[k][:, msl],
                                         rhs=qT[k][:, sl],
                                         start=(k == 0), stop=(k == DT - 1))
                    nc.scalar.activation(out=edt, in_=psd, func=AF.Exp,
                                         bias=bdt[:, m:m + 1])
                    nc.scalar.activation(out=spdt[m][:, sl], in_=edt, func=AF.Ln,
                                         bias=onef)
                    nc.vector.tensor_mul(u[m][:, sl], spdt[m][:, sl],
                                         qT[m][:, sl])

            def knat_tr():
                # k natural layout [i, dk] via PE transposes of kT (the k
                # projection already includes its bias)
                for i in range(TTN):
                    isl = slice(i * 128, (i + 1) * 128)
                    pst = psY.tile([128, 512], F16, tag="tr", name="tr")
                    for k in range(DT):
                        nc.tensor.transpose(pst[:, k * 128:(k + 1) * 128],
                                            kT[k][:, isl], idt)
                    nc.vector.tensor_copy(out=k_nat[i], in_=pst)

            def attn_chunk(c, wbc, mid_hook=None):
                sl = slice(c * 512, (c + 1) * 512)
                # scores directly transposed: [i-part, t-free]
                for i in range(TTN):
                    isl = slice(i * 128, (i + 1) * 128)
                    psx = psA.tile([128, 512], F32, tag="ps", name="qk")
                    for k in range(DT):
                        nc.tensor.matmul(psx, lhsT=kT[k][:, isl],
                                         rhs=qT[k][:, sl],
                                         start=(k == 0), stop=(k == DT - 1))
                    nc.scalar.activation(out=attnT[i], in_=psx, func=AF.Exp,
                                         scale=float(SCALE))
                if mid_hook is not None:
                    mid_hook()
                # softmax denominator, replicated onto 2N partitions
                psr = psR.tile([2 * N, 512], F32, tag="rs", name="rs")
                for i in range(TTN):
                    nc.tensor.matmul(psr, lhsT=ones32, rhs=attnT[i],
                                     start=(i == 0), stop=(i == TTN - 1))
                rsr = sm.tile([2 * N, 512], F16, tag="rsr", name="rsr", bufs=1)
                with nc.allow_low_precision(reason="1/softmax-denominator in f16 is ~5e-4 rel"):
                    nc.vector.reciprocal(rsr, psr)
                # B_feat (unnormalized)
                for m in range(DT):
                    msl = slice(m * 128, (m + 1) * 128)
                    ps = psA.tile([128, 512], F32, tag="ps", name="bf")
                    for i in range(TTN):
                        nc.tensor.matmul(ps, lhsT=k_nat[i][:, msl],
                                         rhs=attnT[i],
                                         start=(i == 0), stop=(i == TTN - 1))
                    nc.vector.tensor_copy(out=bfT[m], in_=ps)
                # B/C projection, scaled by the softmax denominator
                ps = psA.tile([2 * N, 512], F32, tag="ps", name="bc")
                for k in range(DT):
                    nc.tensor.matmul(ps, lhsT=wbc[k], rhs=bfT[k],
                                     start=(k == 0), stop=(k == DT - 1))
                nc.vector.tensor_mul(bc_nT[:, sl], ps, rsr)
                if not zero_bias:
                    nc.vector.tensor_scalar(out=bc_nT[:, sl], in0=bc_nT[:, sl],
                                            scalar1=bbc[:, 0:1], scalar2=None,
                                            op0=OP.add)
                nc.sync.dma_start(out=bc_dram[:, sl], in_=bc_nT[:, sl])

            def bc_broadcast(c):
                sl = slice(c * 512, (c + 1) * 512)
                nc.sync.dma_start(
                    out=Bb[c],
                    in_=bc_dram[0:N, sl].unsqueeze(0).broadcast_to([128, N, TC]))
                nc.sync.dma_start(
                    out=Cb[c],
                    in_=bc_dram[N:2 * N, sl].unsqueeze(0).broadcast_to([128, N, TC]))

            def dA_exp(c, ms, scp):
                sl = slice(c * TC, (c + 1) * TC)
                for m in ms:
                    dA = scp.tile([128, N, TC], F16, tag="dA", name="dA", bufs=3)
                    pre_dA[(c, m)] = dA
                    for n in range(N):
                        nc.scalar.activation(
                            out=dA[:, n, :], in_=spdt[m][:, sl], func=AF.Exp,
                            scale=acols[:, m * N + n:m * N + n + 1])

            def scan_chunk(c, scp, hook=None):
                sl = slice(c * TC, (c + 1) * TC)
                for m in range(DT):
                    if hook is not None:
                        hook(m)
                    dA = pre_dA.pop((c, m))
                    dBx = scp.tile([128, N, TC], F16, tag="dBx", name="dBx",
                                   bufs=3)
                    uv = u[m][:, sl].unsqueeze(1).broadcast_to([128, N, TC])
                    # split row-wise between DVE (2x mode) and GpSimd so both
                    # finish together
                    nc.vector.tensor_mul(dBx[:, :NS], uv[:, :NS], Bb[c][:, :NS])
                    nc.gpsimd.tensor_mul(dBx[:, NS:], uv[:, NS:], Bb[c][:, NS:])
                    if c == 0:
                        nc.vector.memset(dA[:, :, 0:1], 0.0)
                    else:
                        tmp = sm.tile([128, N, 1], F16, tag="carry", name="carry")
                        nc.vector.tensor_mul(tmp, dA[:, :, 0:1], hcar[m])
                        nc.vector.tensor_add(dBx[:, :, 0:1], dBx[:, :, 0:1], tmp)
                        nc.vector.memset(dA[:, :, 0:1], 0.0)
                    # scan in place: h overwrites dA (dA's last access -> its
                    # buffer frees at scan end, keeping the ACT exp pipeline
                    # ahead of DVE)
                    nc.vector.tensor_tensor_scan(
                        out=dA.rearrange("p n t -> p (n t)"),
                        data0=dA.rearrange("p n t -> p (n t)"),
                        data1=dBx.rearrange("p n t -> p (n t)"),
                        initial=0.0, op0=OP.mult, op1=OP.add)
                    if c < NCH - 1:
                        nc.vector.tensor_copy(out=hcar[m], in_=dA[:, :, TC - 1:TC])
                    # hC overwrites the dead dBx tile, not h: frees dA early
                    nc.vector.tensor_mul(dBx[:, :YS], dA[:, :YS], Cb[c][:, :YS])
                    nc.gpsimd.tensor_mul(dBx[:, YS:], dA[:, YS:], Cb[c][:, YS:])
                    psy = psY.tile([128, TC], F32, tag="ys", name="ys")
                    for n in range(N):
                        nc.tensor.matmul(psy, lhsT=idt, rhs=dBx[:, n, :],
                                         start=(n == 0), stop=(n == N - 1))
                    nc.scalar.copy(out=yT[m][:, sl], in_=psy)

            ln_pend = []

            def outproj_chunk(c, op, gam, bet, epst):
                for t in range(4 * c, 4 * c + 4):
                    tsl = slice(t * 128, (t + 1) * 128)
                    ps = psA.tile([128, D], F32, tag="ps", name=f"out{t}")
                    for k in range(DT):
                        nc.tensor.matmul(ps, lhsT=x_s[k][:, tsl], rhs=wout[k],
                                         start=(k == 0), stop=False)
                    for k in range(DT):
                        nc.tensor.matmul(ps, lhsT=yT[k][:, tsl], rhs=wout[DT + k],
                                         start=False,
                                         stop=(zero_bias and k == DT - 1))
                    if not zero_bias:
                        nc.tensor.matmul(ps, lhsT=ones_r, rhs=bout, start=False,
                                         stop=True)
                    outs = op.tile([128, D], F16, tag="outs", name=f"outs{t}",
                                   bufs=4)
                    sx = op.tile([128, 1], F32, tag=f"sx{t}", name=f"sx{t}", bufs=1)
                    nc.scalar.activation(out=outs, in_=ps, func=AF.Copy,
                                         accum_out=sx)
                    sq = op.tile([128, D], F16, tag="sq", name="sq", bufs=1)
                    sx2 = op.tile([128, 1], F32, tag=f"sx2{t}", name=f"sx2{t}",
                                  bufs=1)
                    nc.scalar.activation(out=sq, in_=outs, func=AF.Square,
                                         accum_out=sx2)
                    ln_pend.append((t, outs, sx, sx2))

            def ln_finish(op, gam, bet, epst):
                # mean = sx/D; var = sx2/D - mean^2 (benign cancellation);
                # rstd = exp(-0.5*ln(var+eps))
                pend, ln_pend[:] = list(ln_pend), []
                for t, outs, sx, sx2 in pend:
                    mean = op.tile([128, 1], F32, tag="mean", name="mean")
                    nc.vector.tensor_scalar(out=mean, in0=sx,
                                            scalar1=1.0 / D, scalar2=None,
                                            op0=OP.mult)
                    m2 = op.tile([128, 1], F32, tag="m2", name="m2")
                    nc.vector.tensor_mul(m2, mean, mean)
                    var = op.tile([128, 1], F32, tag="var", name="var")
                    nc.vector.tensor_scalar(out=var, in0=sx2,
                                            scalar1=1.0 / D, scalar2=m2,
                                            op0=OP.mult, op1=OP.subtract)
                    rstd = op.tile([128, 1], F32, tag="rstd", name="rstd")
                    nc.scalar.activation(out=rstd, in_=var, func=AF.Ln,
                                         bias=epst)
                    nc.scalar.activation(out=rstd, in_=rstd, func=AF.Exp,
                                         scale=-0.5)
                    norm = op.tile([128, D], F16, tag="norm", name="norm",
                                   bufs=2)
                    nc.vector.tensor_scalar(out=norm, in0=outs,
                                            scalar1=mean, scalar2=rstd,
                                            op0=OP.subtract, op1=OP.mult)
                    if not unit_ln:
                        nc.gpsimd.tensor_mul(norm, norm, gam)
                        nc.gpsimd.tensor_add(norm, norm, bet)
                    tsl = slice(t * 128, (t + 1) * 128)
                    nc.sync.dma_start(out=out_d[tsl, :], in_=norm)

            # =================== program ===================
            bc_dram = dramp.tile([2 * N, L], F16)

            with (
                tc.tile_pool(name="scp", bufs=1) as scp,
                tc.tile_pool(name="op", bufs=4) as op,
            ):
                hcar = [scp.tile([128, N, 1], F16, tag=f"hcar{m}", name=f"hcar{m}")
                        for m in range(DT)]
                Bb = [scp.tile([128, N, TC], F16, tag=f"Bb{c}", name=f"Bb{c}")
                      for c in range(NCH)]
                Cb = [scp.tile([128, N, TC], F16, tag="Cb", name=f"Cb{c}")
                      for c in range(NCH)]
                if not unit_ln:
                    gam = op.tile([128, D], F32, tag="gam", name="gam", bufs=1)
                    bet = op.tile([128, D], F32, tag="bet", name="bet", bufs=1)
                else:
                    gam = bet = None
                epst = op.tile([128, 1], F32, tag="eps", name="eps", bufs=1)
                nc.vector.memset(epst, EPS)

                with tc.tile_pool(name="phA", bufs=1) as phA, \
                        tc.tile_pool(name="ph1", bufs=1) as ph1:
                    qT = [phA.tile([128, L], F16, tag=f"qT{m}", name=f"qT{m}") for m in range(DT)]
                    kT = [phA.tile([128, L], F16, tag=f"kT{m}", name=f"kT{m}") for m in range(DT)]
                    k_nat = [phA.tile([128, D], F16, tag=f"kn{i}", name=f"kn{i}") for i in range(TTN)]
                    # attnT[i]: unnormalized exp(scores) laid out [i-part,
                    # t-free], one 512-wide chunk slice at a time
                    attnT = [phA.tile([128, TC], F16, tag=f"aT{i}", name=f"aT{i}")
                             for i in range(TTN)]
                    bfT = [phA.tile([128, TC], F16, tag=f"bf{m}", name=f"bf{m}")
                           for m in range(DT)]
                    bc_nT = phA.tile([2 * N, L], F16, tag="bcnT", name="bcnT")
                    x_o = [ph1.tile([128, L], F16, tag=f"xo{m}", name=f"xo{m}")
                           for m in range(DT)]
                    for m in range(DT):
                        nc.sync.dma_start(out=x_o[m], in_=xo_d[m * 128:(m + 1) * 128, :])
                    wq = [ph1.tile([128, D], F16, tag=f"wq{k}", name=f"wq{k}") for k in range(DT)]
                    wk = [ph1.tile([128, D], F16, tag=f"wk{k}", name=f"wk{k}") for k in range(DT)]
                    wdt = [ph1.tile([128, D], F16, tag=f"wdt{k}", name=f"wdt{k}") for k in range(DT)]
                    wbc = [wp.tile([128, 2 * N], F16, tag=f"wbc{k}", name=f"wbc{k}") for k in range(DT)]
                    for k in range(DT):
                        nc.sync.dma_start(out=wk[k], in_=wk_d[k * 128:(k + 1) * 128, :])
                        nc.sync.dma_start(out=wq[k], in_=wq_d[k * 128:(k + 1) * 128, :])
                        nc.sync.dma_start(out=wdt[k], in_=wdt_d[k * 128:(k + 1) * 128, :])
                        nc.sync.dma_start(out=wbc[k], in_=wbc_d[k * 128:(k + 1) * 128, :])
                    _load_constants()
                    if not unit_ln:
                        nc.sync.dma_start(out=gam, in_=gam_d[0:1, :].broadcast_to([128, D]))
                        nc.sync.dma_start(out=bet, in_=bet_d[0:1, :].broadcast_to([128, D]))
                    for k in range(2 * DT):
                        nc.sync.dma_start(out=wout[k],
                                          in_=wout_d[k * 128:(k + 1) * 128, :])

                    proj_half(kT, wk, x_o, bk, 0)
                    proj_half(kT, wk, x_o, bk, 1)
                    proj_half(qT, wq, x_s, bq, 0)
                    attn_chunk(0, wbc,
                               mid_hook=lambda: (dt_half(ph1, wdt, 0),
                                                 knat_tr()))
                    bc_broadcast(0)
                    dA_exp(0, [0], scp)
                    proj_half(qT, wq, x_s, bq, 1)
                    attn_chunk(1, wbc)
                    bc_broadcast(1)
                    dt_half(ph1, wdt, 1)
                    dA_exp(0, [1], scp)

                with tc.tile_pool(name="scp2", bufs=1) as scp2:
                    def hook0(m):
                        if m < 2:
                            dA_exp(0, [m + 2], scp)
                        else:
                            dA_exp(1, [m - 2], scp)

                    def hook1(m):
                        if m < 2:
                            dA_exp(1, [m + 2], scp)

                    scan_chunk(0, scp2, hook=hook0)
                    outproj_chunk(0, op, gam, bet, epst)
                    scan_chunk(1, scp2, hook=hook1)
                    ln_finish(op, gam, bet, epst)
                    outproj_chunk(1, op, gam, bet, epst)
                    ln_finish(op, gam, bet, epst)
    nc.finalize()
    return nc


def _make_runner(nc, n_cores):
    import jax
    from concourse import bass2jax

    bass2jax.install_neuronx_cc_hook()
    partition_name = nc.partition_id_tensor.name if nc.partition_id_tensor else None
    in_names, out_names, out_avals, zero_outs = [], [], [], []
    for alloc in nc.m.functions[0].allocations:
        if not isinstance(alloc, mybir.MemoryLocationSet):
            continue
        name = alloc.memorylocations[0].name
        if alloc.kind == "ExternalInput":
            if name != partition_name:
                in_names.append(name)
        elif alloc.kind == "ExternalOutput":
            shape = tuple(alloc.tensor_shape)
            dtype = mybir.dt.np(alloc.dtype)
            out_names.append(name)
            out_avals.append(jax.core.ShapedArray(shape, dtype))
            zero_outs.append(np.zeros(shape, dtype))
    all_in_names = list(in_names) + list(out_names)
    if partition_name is not None:
        all_in_names.append(partition_name)

    def _body(*args):
        operands = list(args)
        if partition_name is not None:
            operands.append(bass2jax.partition_id_tensor())
        outs = bass2jax._bass_exec_p.bind(
            *operands,
            out_avals=tuple(out_avals),
            in_names=tuple(all_in_names),
            out_names=tuple(out_names),
            lowering_input_output_aliases=(),
            sim_require_finite=False,
            sim_require_nnan=False,
            nc=nc,
        )
        return tuple(outs)

    if n_cores == 1:
        jit_body = jax.jit(_body, keep_unused=True)

        def run(in_maps):
            args = [np.asarray(in_maps[0][n]) for n in in_names] + zero_outs
            outs = jit_body(*args)
            return [dict(zip(out_names, [np.asarray(o) for o in outs]))]
        return run

    from jax.sharding import PartitionSpec as P
    from jax.experimental.shard_map import shard_map
    mesh = jax.make_mesh((n_cores,), ("core",), devices=jax.devices()[:n_cores])
    smapped = jax.jit(
        shard_map(_body, mesh=mesh, in_specs=P("core"), out_specs=P("core"),
                  check_rep=False),
        keep_unused=True)

    def run(in_maps):
        assert len(in_maps) == n_cores
        args = [np.concatenate([np.asarray(m[n]) for m in in_maps], axis=0)
                for n in in_names]
        args += [np.concatenate([z] * n_cores, axis=0) for z in zero_outs]
        outs = [np.asarray(o) for o in smapped(*args)]
        percore = []
        for cidx in range(n_cores):
            d = {}
            for name, o in zip(out_names, outs):
                rows = o.shape[0] // n_cores
                d[name] = o[cidx * rows:(cidx + 1) * rows]
            percore.append(d)
        return percore
    return run


def _core_inputs(x_self, x_other, W_self, b_self, W_other, b_other,
                 W_dt, b_dt, W_B, b_B, W_C, b_C, A, W_out, b_out, gamma, beta):
    f16 = np.float16
    f32 = np.float32
    return {
        "xT_self": np.ascontiguousarray(x_self.T).astype(f16),
        "xT_other": np.ascontiguousarray(x_other.T).astype(f16),
        "w_q": np.ascontiguousarray(W_self[:, :D]).astype(f16),
        "w_k": np.ascontiguousarray(W_other[:, D:]).astype(f16),
        "w_dt": np.ascontiguousarray(W_dt).astype(f16),
        "w_bc": np.concatenate([W_B, W_C], axis=1).astype(f16),
        "w_out": np.ascontiguousarray(W_out).astype(f16),
        "a_cols": np.ascontiguousarray(
            A.reshape(DT, 128, N).transpose(1, 0, 2).reshape(128, DT * N)).astype(f32),
        "b_q": np.ascontiguousarray(b_self[:D].reshape(DT, 128).T).astype(f32),
        "b_dt": np.ascontiguousarray(b_dt.reshape(DT, 128).T).astype(f32),
        "b_k": np.ascontiguousarray(b_other[D:].reshape(DT, 128).T).astype(f32),
        "b_k_row": b_other[D:].reshape(1, D).astype(f16),
        "b_bc": np.concatenate([b_B, b_C]).reshape(2 * N, 1).astype(f32),
        "b_out_row": b_out.reshape(1, D).astype(f16),
        "gamma_row": gamma.reshape(1, D).astype(f32),
        "beta_row": beta.reshape(1, D).astype(f32),
        "ones_row": np.ones((1, 128), f16),
        "ident": np.eye(128, dtype=f16),
    }


def kernel(text_feats, image_feats,
           W_t2i, b_t2i, W_i2t, b_i2t,
           W_dt_t2i, b_dt_t2i, W_dt_i2t, b_dt_i2t,
           W_B_t2i, b_B_t2i, W_B_i2t, b_B_i2t,
           W_C_t2i, b_C_t2i, W_C_i2t, b_C_i2t,
           A_log, W_out, b_out, gamma, beta):
    text_feats = np.asarray(text_feats, np.float32)
    image_feats = np.asarray(image_feats, np.float32)
    A = -np.exp(np.asarray(A_log, np.float32))

    zero_bias = all(not np.any(np.asarray(x)) for x in
                    (b_t2i, b_i2t, b_dt_t2i, b_dt_i2t, b_B_t2i, b_B_i2t,
                     b_C_t2i, b_C_i2t, b_out))
    unit_ln = (not np.any(np.asarray(beta))) and np.all(np.asarray(gamma) == 1.0)
    key = ("nc", zero_bias, unit_ln)
    if key not in _CACHE:
        _CACHE[key] = _build_program(zero_bias=zero_bias, unit_ln=unit_ln)
        _CACHE["nc"] = _CACHE[key]
    nc = _CACHE[key]
    n_cores = 8
    if ("runner", key) not in _CACHE:
        _CACHE[("runner", key)] = _make_runner(nc, n_cores)
    run = _CACHE[("runner", key)]

    in_maps = []
    for b in range(B):
        # direction 0: t2i (text output)
        in_maps.append(_core_inputs(
            text_feats[b], image_feats[b], W_t2i, b_t2i, W_i2t, b_i2t,
            W_dt_t2i, b_dt_t2i, W_B_t2i, b_B_t2i, W_C_t2i, b_C_t2i,
            A, W_out, b_out, gamma, beta))
        # direction 1: i2t (image output)
        in_maps.append(_core_inputs(
            image_feats[b], text_feats[b], W_i2t, b_i2t, W_t2i, b_t2i,
            W_dt_i2t, b_dt_i2t, W_B_i2t, b_B_i2t, W_C_i2t, b_C_i2t,
            A, W_out, b_out, gamma, beta))

    results = run(in_maps)
    text_out = np.stack([results[2 * b]["out"] for b in range(B)],
                        axis=0).astype(np.float32)
    image_out = np.stack([results[2 * b + 1]["out"] for b in range(B)],
                         axis=0).astype(np.float32)
    return text_out, image_out
